# revision 20
# baseline (speedup 1.0000x reference)
"""Trainium2 Bass kernel for additive-attention pooling.

Reference math (per sample b):
    score  = tanh(x @ W_w + W_b)          # [T, U]
    logits = score @ V_w + V_b            # [T, 1]
    attn   = softmax(logits, axis=T)
    out    = sum_t attn[t] * x[t, :]      # [D]

Shapes: x [64, 4096, 256] f32, W_w [256, 256], W_b [256], V_w [256, 1], V_b [1].
V_b shifts every logit of a sample equally, so it cancels in the softmax.

Strategy: data-parallel over batch, 8 samples per core on 8 NeuronCores.
  - GEMM1 (TensorE, bf16) computes score TRANSPOSED ([u, t] layout):
    lhsT = W block, rhs = xT. The host ships x pre-transposed so no on-chip
    transpose is needed.
  - tanh on ScalarE (PSUM -> SBUF, per-partition bias = W_b chunk).
  - V-dot split: VectorE pre-reduces the two u-halves with per-partition
    scalars (z = V0*tanh0 + V1*tanh1), then TensorE reduces over the
    remaining 128 partitions with a single-matmul group per 128-row chunk
    (lhsT = z chunk, rhs = ones): logits land [t, 1]-shaped, exactly the
    layout the weighted sum needs for its stationary operand. This keeps
    the expensive stationary-switch count on TensorE low.
  - exp on ScalarE, batched once per sample ([128, 32]).
  - weighted sum (TensorE): lhsT = w [t, 1], rhs = x natural (host also
    ships x in natural layout, bf16, with a ones column appended so the
    softmax denominator falls out of the same matmul). Lags one sample
    behind the GEMM pipeline so it never waits on exp.
Softmax max-subtraction is skipped: |logit| <= sum|V| < 20, safely in fp32/bf16
exp range.
"""

import numpy as np
import ml_dtypes

# ---- problem constants (hardcoded; kernel.py must be self-contained) ----
B, T, D, U = 64, 4096, 256, 256
N_CORES = 8
S = B // N_CORES          # samples per core
TT = 512                  # t-tile (rows per pipeline step)
N_TILES = T // TT         # tiles per sample (8)
CH = TT // 128            # 128-row chunks per tile (4)
LAG_L2 = 2                # V-dot lag in tiles (z computed on VectorE)
LAG_W = 4                 # weighted-sum lag in tiles (exp per 2 tiles)

BF16 = ml_dtypes.bfloat16

_CACHE = {}


def _build():
    import concourse.bass as bass
    import concourse.tile as tile
    from concourse import bacc, mybir
    from concourse.bass import ds, ts

    f32 = mybir.dt.float32
    bf16 = mybir.dt.bfloat16
    Tanh = mybir.ActivationFunctionType.Tanh
    Exp = mybir.ActivationFunctionType.Exp

    nc = bacc.Bacc("TRN2", target_bir_lowering=False, debug=False)

    xT_d = nc.dram_tensor("xT", [S, D, T], bf16, kind="ExternalInput").ap()
    xn_d = nc.dram_tensor("xn", [S, T, D + 1], bf16, kind="ExternalInput").ap()
    w_d = nc.dram_tensor("w", [D, U], bf16, kind="ExternalInput").ap()
    wb_d = nc.dram_tensor("wb", [128, U // 128], f32, kind="ExternalInput").ap()
    v_d = nc.dram_tensor("v", [128, U // 128], f32, kind="ExternalInput").ap()
    out_d = nc.dram_tensor("out", [S, D], f32, kind="ExternalOutput").ap()

    NG = S * N_TILES  # total pipeline steps (64)

    with tile.TileContext(nc) as tc:
        with (
            tc.tile_pool(name="const", bufs=1) as const_pool,
            tc.tile_pool(name="xT", bufs=10) as xT_pool,
            tc.tile_pool(name="xn", bufs=LAG_W + 4) as xn_pool,
            tc.tile_pool(name="tanh", bufs=3) as tanh_pool,
            tc.tile_pool(name="z", bufs=6) as z_pool,
            tc.tile_pool(name="wexp", bufs=2) as wexp_pool,
            tc.tile_pool(name="fin", bufs=2) as fin_pool,
            tc.tile_pool(name="score_ps", bufs=6, space="PSUM") as score_pool,
            tc.tile_pool(name="logit_ps", bufs=1, space="PSUM") as logit_pool,
            tc.tile_pool(name="c_ps", bufs=1, space="PSUM") as c_pool,
        ):
            # constants
            w_sb = const_pool.tile([128, 2, U], bf16)     # [d_in_chunk, d_chunk, u]
            nc.gpsimd.dma_start(w_sb[:], w_d.rearrange("(k p) u -> p k u", p=128))
            v_sb = const_pool.tile([128, 2], f32)         # [u_in_chunk, u_chunk]
            nc.gpsimd.dma_start(v_sb[:], v_d)
            wb_sb = const_pool.tile([128, 2], f32)
            nc.gpsimd.dma_start(wb_sb[:], wb_d)
            ones_sb = const_pool.tile([128, 1], bf16)
            nc.vector.memset(ones_sb[:], 1.0)

            z_tiles = {}        # g -> V-reduced tanh tile [128, TT]
            score_tiles = {}    # (g, uc) -> psum score tile
            xn_tiles = {}       # g -> xn tile
            logit_tiles = {}    # sample -> [128, N_TILES*CH] psum tile
            wexp_tiles = {}     # sample -> [128, N_TILES*CH] bf16 weights
            c_tiles = {}        # sample -> [1, D+1] psum accumulator

            def emit_l2(j, c):
                """Partition-reduce of z chunk c for tile j -> logit column."""
                sj, ttj = divmod(j, N_TILES)
                nc.tensor.matmul(
                    logit_tiles[sj][:, ds(ttj * CH + c, 1)],
                    z_tiles[j][:, ts(c, 128)],
                    ones_sb[:],
                    start=True,
                    stop=True,
                )
                if c == CH - 1:
                    del z_tiles[j]

            def emit_wsum_chunk(j, c):
                """One 128-row chunk of the weighted sum for tile j."""
                sj, ttj = divmod(j, N_TILES)
                nc.tensor.matmul(
                    c_tiles[sj][:],
                    wexp_tiles[sj][:, ds(ttj * CH + c, 1)],
                    xn_tiles[j][:, c, :],
                    start=(ttj == 0 and c == 0),
                    stop=(ttj == N_TILES - 1 and c == CH - 1),
                )
                if c == CH - 1:
                    del xn_tiles[j]

            for g in range(NG + LAG_W + 1):
                s, tt = divmod(g, N_TILES) if g < NG else (None, None)
                jw = g - LAG_W  # tile index for weighted sum this iteration
                jl = g - LAG_L2  # tile index for V-dot this iteration

                # ---- DMA + paired GEMM (W block stationary reused) ----
                if g < NG and g % 2 == 0:
                    pair = [g, g + 1]
                    xt_pair = []
                    for gg in pair:
                        ss, tts = divmod(gg, N_TILES)
                        xT_t = xT_pool.tile([128, 2, TT], bf16, tag="xT", name=f"xT{gg}")
                        nc.sync.dma_start(
                            xT_t[:],
                            xT_d[ss, :, ts(tts, TT)].rearrange(
                                "(k p) t -> p k t", p=128
                            ),
                        )
                        xt_pair.append(xT_t)
                        xn_t = xn_pool.tile([128, CH, D + 1], bf16, tag="xn", name=f"xn{gg}")
                        nc.gpsimd.dma_start(
                            xn_t[:],
                            xn_d[ss, ts(tts, TT), :].rearrange(
                                "(c p) f -> p c f", p=128
                            ),
                        )
                        xn_tiles[gg] = xn_t
                        if tts == 0:
                            logit_tiles[ss] = logit_pool.tile(
                                [128, N_TILES * CH], f32, tag="logit",
                                name=f"logit{ss}",
                            )
                            c_tiles[ss] = c_pool.tile(
                                [1, D + 1], f32, tag="acc", name=f"acc{ss}"
                            )
                    for uc in range(2):
                        scs = [
                            score_pool.tile(
                                [128, TT], f32, tag="score", name=f"sc{gg}_{uc}"
                            )
                            for gg in pair
                        ]
                        for dc in range(2):
                            for pi in range(2):
                                nc.tensor.matmul(
                                    scs[pi][:],
                                    w_sb[:, dc, ts(uc, 128)],
                                    xt_pair[pi][:, dc, :],
                                    start=(dc == 0),
                                    stop=(dc == 1),
                                )
                        score_tiles[(pair[0], uc)] = scs[0]
                        score_tiles[(pair[1], uc)] = scs[1]

                # ---- tail matmuls: L2 + wsum ----
                li = 0
                n_l2 = CH if 0 <= jl < NG else 0
                if 0 <= jw < NG:
                    for c in range(CH):
                        emit_wsum_chunk(jw, c)
                        if li < n_l2:
                            emit_l2(jl, li)
                            li += 1
                while li < n_l2:
                    emit_l2(jl, li)
                    li += 1

                # ---- ACT: tanh; DVE: fold V into the two u-halves ----
                if g < NG:
                    tanh_t = tanh_pool.tile([128, 2, TT], bf16)
                    for uc in range(2):
                        nc.scalar.activation(
                            tanh_t[:, uc, :],
                            score_tiles.pop((g, uc))[:],
                            Tanh,
                            bias=wb_sb[:, ds(uc, 1)],
                        )
                    q = z_pool.tile([128, TT], bf16, tag="q")
                    nc.vector.tensor_scalar_mul(q[:], tanh_t[:, 0, :], v_sb[:, ds(0, 1)])
                    zt = z_pool.tile([128, TT], bf16, tag="z")
                    nc.vector.tensor_scalar_mul(zt[:], tanh_t[:, 1, :], v_sb[:, ds(1, 1)])
                    nc.vector.tensor_add(zt[:], zt[:], q[:])
                    z_tiles[g] = zt

                # ---- ACT: exp per pair of tiles (quarter-sample batches) ----
                if 0 <= jl < NG and jl % 2 == 1:
                    sj, ttj = divmod(jl, N_TILES)
                    q = ttj // 2
                    if q == 0:
                        wexp_tiles[sj] = wexp_pool.tile(
                            [128, N_TILES * CH], bf16, tag="wexp", name=f"wexp{sj}"
                        )
                    lg = logit_tiles[sj]
                    nc.scalar.activation(
                        wexp_tiles[sj][:, ts(q, 2 * CH)], lg[:, ts(q, 2 * CH)], Exp
                    )
                    if q == N_TILES // 2 - 1:
                        logit_tiles.pop(sj)

                # ---- finalize sample after its last wsum chunk ----
                if 0 <= jw < NG and jw % N_TILES == N_TILES - 1:
                    sj = jw // N_TILES
                    del wexp_tiles[sj]
                    c_ps = c_tiles.pop(sj)
                    recip = fin_pool.tile([1, 1], f32, tag="recip")
                    nc.vector.reciprocal(recip[:], c_ps[0:1, D : D + 1])
                    row = fin_pool.tile([1, D], f32, tag="row")
                    nc.vector.tensor_scalar_mul(row[:], c_ps[0:1, 0:D], recip[:])
                    nc.scalar.dma_start(out_d[ds(sj, 1), :], row[:])

    nc.compile()
    return nc


def _prep_inputs(inputs, W_w, W_b, V_w, V_b):
    x = np.asarray(inputs, dtype=np.float32)
    ones = np.ones((B, T, 1), dtype=np.float32)
    xn_full = np.concatenate([x, ones], axis=2).astype(BF16)      # [B, T, D+1]
    xT_full = np.ascontiguousarray(x.transpose(0, 2, 1)).astype(BF16)  # [B, D, T]

    w = np.asarray(W_w, dtype=np.float32).astype(BF16)            # [D, U]
    wb = np.asarray(W_b, dtype=np.float32).reshape(U // 128, 128).T.copy()  # [128, 2]
    v = np.asarray(V_w, dtype=np.float32).reshape(U // 128, 128).T.copy()  # [128, 2]

    in_maps = []
    for c in range(N_CORES):
        sl = slice(c * S, (c + 1) * S)
        in_maps.append(
            {
                "xT": np.ascontiguousarray(xT_full[sl]),
                "xn": np.ascontiguousarray(xn_full[sl]),
                "w": w,
                "wb": wb,
                "v": v,
            }
        )
    return in_maps


def kernel(inputs, W_w, W_b, V_w, V_b):
    from concourse.bass_utils import run_bass_kernel_spmd

    if "nc" not in _CACHE:
        _CACHE["nc"] = _build()
    nc = _CACHE["nc"]

    in_maps = _prep_inputs(inputs, W_w, W_b, V_w, V_b)
    res = run_bass_kernel_spmd(nc, in_maps, core_ids=list(range(N_CORES)))
    out = np.concatenate([r["out"] for r in res.results], axis=0)
    return np.asarray(out, dtype=np.float32)


# revision 22
# speedup vs baseline: 1.0580x; 1.0580x over previous
"""Trainium2 Bass kernel for additive-attention pooling.

Reference math (per sample b):
    score  = tanh(x @ W_w + W_b)          # [T, U]
    logits = score @ V_w + V_b            # [T, 1]
    attn   = softmax(logits, axis=T)
    out    = sum_t attn[t] * x[t, :]      # [D]

Shapes: x [64, 4096, 256] f32, W_w [256, 256], W_b [256], V_w [256, 1], V_b [1].
V_b shifts every logit of a sample equally, so it cancels in the softmax.

Strategy: data-parallel over batch, 8 samples per core on 8 NeuronCores.
  - GEMM1 (TensorE, bf16) computes score TRANSPOSED ([u, t] layout):
    lhsT = W block, rhs = xT. The host ships x pre-transposed so no on-chip
    transpose is needed.
  - tanh on ScalarE (PSUM -> SBUF, per-partition bias = W_b chunk).
  - V-dot split: VectorE pre-reduces the two u-halves with per-partition
    scalars (z = V0*tanh0 + V1*tanh1), then TensorE reduces over the
    remaining 128 partitions with a single-matmul group per 128-row chunk
    (lhsT = z chunk, rhs = ones): logits land [t, 1]-shaped, exactly the
    layout the weighted sum needs for its stationary operand. This keeps
    the expensive stationary-switch count on TensorE low.
  - exp on ScalarE, batched once per sample ([128, 32]).
  - weighted sum (TensorE): lhsT = w [t, 1], rhs = x natural (host also
    ships x in natural layout, bf16, with a ones column appended so the
    softmax denominator falls out of the same matmul). Lags one sample
    behind the GEMM pipeline so it never waits on exp.
Softmax max-subtraction is skipped: |logit| <= sum|V| < 20, safely in fp32/bf16
exp range.
"""

import numpy as np
import ml_dtypes

# ---- problem constants (hardcoded; kernel.py must be self-contained) ----
B, T, D, U = 64, 4096, 256, 256
N_CORES = 8
S = B // N_CORES          # samples per core
TT = 1024                 # t-tile (rows per pipeline step)
N_TILES = T // TT         # tiles per sample (4)
CH = TT // 128            # 128-row chunks per tile (8)
HF = TT // 512            # 512-col matmul slices per tile (2)
LAG_L2 = 2                # V-dot lag in tiles (z computed on VectorE)
LAG_W = N_TILES + 2       # weighted-sum lag in tiles

BF16 = ml_dtypes.bfloat16

_CACHE = {}


def _build():
    import concourse.bass as bass
    import concourse.tile as tile
    from concourse import bacc, mybir
    from concourse.bass import ds, ts

    f32 = mybir.dt.float32
    bf16 = mybir.dt.bfloat16
    Tanh = mybir.ActivationFunctionType.Tanh
    Exp = mybir.ActivationFunctionType.Exp

    nc = bacc.Bacc("TRN2", target_bir_lowering=False, debug=False)

    xT_d = nc.dram_tensor("xT", [S, D, T], bf16, kind="ExternalInput").ap()
    xn_d = nc.dram_tensor("xn", [S, T, D + 1], bf16, kind="ExternalInput").ap()
    w_d = nc.dram_tensor("w", [D, U], bf16, kind="ExternalInput").ap()
    wb_d = nc.dram_tensor("wb", [128, U // 128], f32, kind="ExternalInput").ap()
    v_d = nc.dram_tensor("v", [128, U // 128], f32, kind="ExternalInput").ap()
    out_d = nc.dram_tensor("out", [S, D], f32, kind="ExternalOutput").ap()

    NG = S * N_TILES  # total pipeline steps (64)

    with tile.TileContext(nc) as tc:
        with (
            tc.tile_pool(name="const", bufs=1) as const_pool,
            tc.tile_pool(name="xT", bufs=6) as xT_pool,
            tc.tile_pool(name="xn", bufs=LAG_W + 3) as xn_pool,
            tc.tile_pool(name="tanh", bufs=3) as tanh_pool,
            tc.tile_pool(name="z", bufs=6) as z_pool,
            tc.tile_pool(name="wexp", bufs=2) as wexp_pool,
            tc.tile_pool(name="fin", bufs=2) as fin_pool,
            tc.tile_pool(name="score_ps", bufs=3, space="PSUM") as score_pool,
            tc.tile_pool(name="logit_ps", bufs=1, space="PSUM") as logit_pool,
            tc.tile_pool(name="c_ps", bufs=1, space="PSUM") as c_pool,
        ):
            # constants
            w_sb = const_pool.tile([128, 2, U], bf16)     # [d_in_chunk, d_chunk, u]
            nc.gpsimd.dma_start(w_sb[:], w_d.rearrange("(k p) u -> p k u", p=128))
            v_sb = const_pool.tile([128, 2], f32)         # [u_in_chunk, u_chunk]
            nc.gpsimd.dma_start(v_sb[:], v_d)
            wb_sb = const_pool.tile([128, 2], f32)
            nc.gpsimd.dma_start(wb_sb[:], wb_d)
            ones_sb = const_pool.tile([128, 1], bf16)
            nc.vector.memset(ones_sb[:], 1.0)

            z_tiles = {}        # g -> V-reduced tanh tile [128, TT]
            score_tiles = {}    # (g, uc) -> psum score tile
            xn_tiles = {}       # g -> xn tile
            logit_tiles = {}    # sample -> [128, N_TILES*CH] psum tile
            wexp_tiles = {}     # sample -> [128, N_TILES*CH] bf16 weights
            c_tiles = {}        # sample -> [1, D+1] psum accumulator

            def emit_l2(j, c):
                """Partition-reduce of z chunk c for tile j -> logit column."""
                sj, ttj = divmod(j, N_TILES)
                nc.tensor.matmul(
                    logit_tiles[sj][:, ds(ttj * CH + c, 1)],
                    z_tiles[j][:, ts(c, 128)],
                    ones_sb[:],
                    start=True,
                    stop=True,
                )
                if c == CH - 1:
                    del z_tiles[j]

            def emit_wsum_chunk(j, c):
                """One 128-row chunk of the weighted sum for tile j."""
                sj, ttj = divmod(j, N_TILES)
                nc.tensor.matmul(
                    c_tiles[sj][:],
                    wexp_tiles[sj][:, ds(ttj * CH + c, 1)],
                    xn_tiles[j][:, c, :],
                    start=(ttj == 0 and c == 0),
                    stop=(ttj == N_TILES - 1 and c == CH - 1),
                )
                if c == CH - 1:
                    del xn_tiles[j]

            for g in range(NG + LAG_W + 1):
                s, tt = divmod(g, N_TILES) if g < NG else (None, None)
                jw = g - LAG_W  # tile index for weighted sum this iteration
                jl = g - LAG_L2  # tile index for V-dot this iteration

                # ---- DMA + GEMM for tile g (W stationary reused across halves) ----
                if g < NG:
                    xT_t = xT_pool.tile([128, 2, TT], bf16, tag="xT", name=f"xT{g}")
                    nc.sync.dma_start(
                        xT_t[:],
                        xT_d[s, :, ts(tt, TT)].rearrange("(k p) t -> p k t", p=128),
                    )
                    xn_t = xn_pool.tile(
                        [128, CH, D + 1], bf16, tag="xn", name=f"xn{g}"
                    )
                    nc.gpsimd.dma_start(
                        xn_t[:],
                        xn_d[s, ts(tt, TT), :].rearrange("(c p) f -> p c f", p=128),
                    )
                    xn_tiles[g] = xn_t
                    if tt == 0:
                        logit_tiles[s] = logit_pool.tile(
                            [128, N_TILES * CH], f32, tag="logit", name=f"logit{s}"
                        )
                        c_tiles[s] = c_pool.tile(
                            [1, D + 1], f32, tag="acc", name=f"acc{s}"
                        )
                    for uc in range(2):
                        sc = score_pool.tile(
                            [128, TT], f32, tag="score", name=f"sc{g}_{uc}"
                        )
                        for dc in range(2):
                            for h in range(HF):
                                nc.tensor.matmul(
                                    sc[:, ts(h, 512)],
                                    w_sb[:, dc, ts(uc, 128)],
                                    xT_t[:, dc, ts(h, 512)],
                                    start=(dc == 0),
                                    stop=(dc == 1),
                                )
                        score_tiles[(g, uc)] = sc

                # ---- tail matmuls: L2 + wsum ----
                li = 0
                n_l2 = CH if 0 <= jl < NG else 0
                if 0 <= jw < NG:
                    for c in range(CH):
                        emit_wsum_chunk(jw, c)
                        if li < n_l2:
                            emit_l2(jl, li)
                            li += 1
                while li < n_l2:
                    emit_l2(jl, li)
                    li += 1

                # ---- ACT: tanh; DVE: fold V into the two u-halves ----
                if g < NG:
                    tanh_t = tanh_pool.tile([128, 2, TT], bf16)
                    for uc in range(2):
                        nc.scalar.activation(
                            tanh_t[:, uc, :],
                            score_tiles.pop((g, uc))[:],
                            Tanh,
                            bias=wb_sb[:, ds(uc, 1)],
                        )
                    q = z_pool.tile([128, TT], bf16, tag="q")
                    nc.vector.tensor_scalar_mul(q[:], tanh_t[:, 0, :], v_sb[:, ds(0, 1)])
                    zt = z_pool.tile([128, TT], bf16, tag="z")
                    nc.vector.tensor_scalar_mul(zt[:], tanh_t[:, 1, :], v_sb[:, ds(1, 1)])
                    nc.vector.tensor_add(zt[:], zt[:], q[:])
                    z_tiles[g] = zt

                # ---- ACT: exp once per sample (after last tile's V-dot) ----
                if 0 <= jl < NG and jl % N_TILES == N_TILES - 1:
                    sj = jl // N_TILES
                    lg = logit_tiles.pop(sj)
                    wx = wexp_pool.tile([128, N_TILES * CH], bf16, tag="wexp")
                    nc.scalar.activation(wx[:], lg[:], Exp)
                    wexp_tiles[sj] = wx

                # ---- finalize sample after its last wsum chunk ----
                if 0 <= jw < NG and jw % N_TILES == N_TILES - 1:
                    sj = jw // N_TILES
                    del wexp_tiles[sj]
                    c_ps = c_tiles.pop(sj)
                    recip = fin_pool.tile([1, 1], f32, tag="recip")
                    nc.vector.reciprocal(recip[:], c_ps[0:1, D : D + 1])
                    row = fin_pool.tile([1, D], f32, tag="row")
                    nc.vector.tensor_scalar_mul(row[:], c_ps[0:1, 0:D], recip[:])
                    nc.scalar.dma_start(out_d[ds(sj, 1), :], row[:])

    nc.compile()
    return nc


def _prep_inputs(inputs, W_w, W_b, V_w, V_b):
    x = np.asarray(inputs, dtype=np.float32)
    ones = np.ones((B, T, 1), dtype=np.float32)
    xn_full = np.concatenate([x, ones], axis=2).astype(BF16)      # [B, T, D+1]
    xT_full = np.ascontiguousarray(x.transpose(0, 2, 1)).astype(BF16)  # [B, D, T]

    w = np.asarray(W_w, dtype=np.float32).astype(BF16)            # [D, U]
    wb = np.asarray(W_b, dtype=np.float32).reshape(U // 128, 128).T.copy()  # [128, 2]
    v = np.asarray(V_w, dtype=np.float32).reshape(U // 128, 128).T.copy()  # [128, 2]

    in_maps = []
    for c in range(N_CORES):
        sl = slice(c * S, (c + 1) * S)
        in_maps.append(
            {
                "xT": np.ascontiguousarray(xT_full[sl]),
                "xn": np.ascontiguousarray(xn_full[sl]),
                "w": w,
                "wb": wb,
                "v": v,
            }
        )
    return in_maps


def kernel(inputs, W_w, W_b, V_w, V_b):
    from concourse.bass_utils import run_bass_kernel_spmd

    if "nc" not in _CACHE:
        _CACHE["nc"] = _build()
    nc = _CACHE["nc"]

    in_maps = _prep_inputs(inputs, W_w, W_b, V_w, V_b)
    res = run_bass_kernel_spmd(nc, in_maps, core_ids=list(range(N_CORES)))
    out = np.concatenate([r["out"] for r in res.results], axis=0)
    return np.asarray(out, dtype=np.float32)


# revision 24
# speedup vs baseline: 1.1339x; 1.0717x over previous
"""Trainium2 Bass kernel for additive-attention pooling.

Reference math (per sample b):
    score  = tanh(x @ W_w + W_b)          # [T, U]
    logits = score @ V_w + V_b            # [T, 1]
    attn   = softmax(logits, axis=T)
    out    = sum_t attn[t] * x[t, :]      # [D]

Shapes: x [64, 4096, 256] f32, W_w [256, 256], W_b [256], V_w [256, 1], V_b [1].
V_b shifts every logit of a sample equally, so it cancels in the softmax.

Strategy: data-parallel over batch, 8 samples per core on 8 NeuronCores.
  - GEMM1 (TensorE, bf16) computes score TRANSPOSED ([u, t] layout):
    lhsT = W block, rhs = xT. The host ships x pre-transposed so no on-chip
    transpose is needed.
  - tanh on ScalarE (PSUM -> SBUF, per-partition bias = W_b chunk).
  - V-dot split: VectorE pre-reduces the two u-halves with per-partition
    scalars (z = V0*tanh0 + V1*tanh1), then TensorE reduces over the
    remaining 128 partitions with a single-matmul group per 128-row chunk
    (lhsT = z chunk, rhs = ones): logits land [t, 1]-shaped, exactly the
    layout the weighted sum needs for its stationary operand. This keeps
    the expensive stationary-switch count on TensorE low.
  - exp on ScalarE, batched once per sample ([128, 32]).
  - weighted sum (TensorE): lhsT = w [t, 1], rhs = x natural (host also
    ships x in natural layout, bf16, with a ones column appended so the
    softmax denominator falls out of the same matmul). Lags one sample
    behind the GEMM pipeline so it never waits on exp.
Softmax max-subtraction is skipped: |logit| <= sum|V| < 20, safely in fp32/bf16
exp range.
"""

import numpy as np
import ml_dtypes

# ---- problem constants (hardcoded; kernel.py must be self-contained) ----
B, T, D, U = 64, 4096, 256, 256
N_CORES = 8
S = B // N_CORES          # samples per core
TT = 512                  # t-tile (rows per pipeline step)
N_TILES = T // TT         # tiles per sample (8)
CH = TT // 128            # 128-row chunks per tile (4)
LAG_L2 = 2                # V-dot lag in tiles (z computed on VectorE)
LAG_W = N_TILES + 2       # weighted-sum lag in tiles

BF16 = ml_dtypes.bfloat16

_CACHE = {}


def _build():
    import concourse.bass as bass
    import concourse.tile as tile
    from concourse import bacc, mybir
    from concourse.bass import ds, ts

    f32 = mybir.dt.float32
    bf16 = mybir.dt.bfloat16
    Tanh = mybir.ActivationFunctionType.Tanh
    Exp = mybir.ActivationFunctionType.Exp

    nc = bacc.Bacc("TRN2", target_bir_lowering=False, debug=False)

    xT_d = nc.dram_tensor("xT", [S, D, T], bf16, kind="ExternalInput").ap()
    xn_d = nc.dram_tensor("xn", [S, T, D + 1], bf16, kind="ExternalInput").ap()
    w_d = nc.dram_tensor("w", [D, U], bf16, kind="ExternalInput").ap()
    wb_d = nc.dram_tensor("wb", [128, U // 128], f32, kind="ExternalInput").ap()
    v_d = nc.dram_tensor("v", [128, U // 128], f32, kind="ExternalInput").ap()
    out_d = nc.dram_tensor("out", [S, D], f32, kind="ExternalOutput").ap()

    NG = S * N_TILES  # total pipeline steps (64)

    with tile.TileContext(nc) as tc:
        with (
            tc.tile_pool(name="const", bufs=1) as const_pool,
            tc.tile_pool(name="xT", bufs=10) as xT_pool,
            tc.tile_pool(name="xn", bufs=LAG_W + 3) as xn_pool,
            tc.tile_pool(name="tanh", bufs=3) as tanh_pool,
            tc.tile_pool(name="z", bufs=6) as z_pool,
            tc.tile_pool(name="wexp", bufs=2) as wexp_pool,
            tc.tile_pool(name="fin", bufs=2) as fin_pool,
            tc.tile_pool(name="score_ps", bufs=6, space="PSUM") as score_pool,
            tc.tile_pool(name="logit_ps", bufs=1, space="PSUM") as logit_pool,
            tc.tile_pool(name="c_ps", bufs=1, space="PSUM") as c_pool,
        ):
            # constants
            w_sb = const_pool.tile([128, 2, U], bf16)     # [d_in_chunk, d_chunk, u]
            nc.gpsimd.dma_start(w_sb[:], w_d.rearrange("(k p) u -> p k u", p=128))
            v_sb = const_pool.tile([128, 2], f32)         # [u_in_chunk, u_chunk]
            nc.gpsimd.dma_start(v_sb[:], v_d)
            wb_sb = const_pool.tile([128, 2], f32)
            nc.gpsimd.dma_start(wb_sb[:], wb_d)
            ones_sb = const_pool.tile([128, 1], bf16)
            nc.vector.memset(ones_sb[:], 1.0)

            z_tiles = {}        # g -> V-reduced tanh tile [128, TT]
            score_tiles = {}    # (g, uc) -> psum score tile
            xn_tiles = {}       # g -> xn tile
            logit_tiles = {}    # sample -> [128, N_TILES*CH] psum tile
            wexp_tiles = {}     # sample -> [128, N_TILES*CH] bf16 weights
            c_tiles = {}        # sample -> [1, D+1] psum accumulator

            def emit_l2(j, c):
                """Partition-reduce of z chunk c for tile j -> logit column."""
                sj, ttj = divmod(j, N_TILES)
                nc.tensor.matmul(
                    logit_tiles[sj][:, ds(ttj * CH + c, 1)],
                    z_tiles[j][:, ts(c, 128)],
                    ones_sb[:],
                    start=True,
                    stop=True,
                )
                if c == CH - 1:
                    del z_tiles[j]

            def emit_wsum_chunk(j, c):
                """One 128-row chunk of the weighted sum for tile j."""
                sj, ttj = divmod(j, N_TILES)
                nc.tensor.matmul(
                    c_tiles[sj][:],
                    wexp_tiles[sj][:, ds(ttj * CH + c, 1)],
                    xn_tiles[j][:, c, :],
                    start=(ttj == 0 and c == 0),
                    stop=(ttj == N_TILES - 1 and c == CH - 1),
                )
                if c == CH - 1:
                    del xn_tiles[j]

            for g in range(NG + LAG_W + 1):
                s, tt = divmod(g, N_TILES) if g < NG else (None, None)
                jw = g - LAG_W  # tile index for weighted sum this iteration
                jl = g - LAG_L2  # tile index for V-dot this iteration

                # ---- DMA + paired GEMM (W block stationary reused) ----
                if g < NG and g % 2 == 0:
                    pair = [g, g + 1]
                    xt_pair = []
                    for gg in pair:
                        ss, tts = divmod(gg, N_TILES)
                        xT_t = xT_pool.tile([128, 2, TT], bf16, tag="xT", name=f"xT{gg}")
                        nc.sync.dma_start(
                            xT_t[:],
                            xT_d[ss, :, ts(tts, TT)].rearrange(
                                "(k p) t -> p k t", p=128
                            ),
                        )
                        xt_pair.append(xT_t)
                        xn_t = xn_pool.tile([128, CH, D + 1], bf16, tag="xn", name=f"xn{gg}")
                        nc.gpsimd.dma_start(
                            xn_t[:],
                            xn_d[ss, ts(tts, TT), :].rearrange(
                                "(c p) f -> p c f", p=128
                            ),
                        )
                        xn_tiles[gg] = xn_t
                        if tts == 0:
                            logit_tiles[ss] = logit_pool.tile(
                                [128, N_TILES * CH], f32, tag="logit",
                                name=f"logit{ss}",
                            )
                            c_tiles[ss] = c_pool.tile(
                                [1, D + 1], f32, tag="acc", name=f"acc{ss}"
                            )
                    for uc in range(2):
                        scs = [
                            score_pool.tile(
                                [128, TT], f32, tag="score", name=f"sc{gg}_{uc}"
                            )
                            for gg in pair
                        ]
                        for dc in range(2):
                            for pi in range(2):
                                nc.tensor.matmul(
                                    scs[pi][:],
                                    w_sb[:, dc, ts(uc, 128)],
                                    xt_pair[pi][:, dc, :],
                                    start=(dc == 0),
                                    stop=(dc == 1),
                                )
                        score_tiles[(pair[0], uc)] = scs[0]
                        score_tiles[(pair[1], uc)] = scs[1]

                # ---- tail matmuls: L2 + wsum in alternating pairs ----
                li = 0
                n_l2 = CH if 0 <= jl < NG else 0
                if 0 <= jw < NG:
                    for c in range(0, CH, 2):
                        emit_wsum_chunk(jw, c)
                        emit_wsum_chunk(jw, c + 1)
                        while li < min(n_l2, li + 2):
                            emit_l2(jl, li)
                            li += 1
                            if li % 2 == 0:
                                break
                while li < n_l2:
                    emit_l2(jl, li)
                    li += 1

                # ---- ACT: tanh; DVE: fold V into the two u-halves ----
                if g < NG:
                    tanh_t = tanh_pool.tile([128, 2, TT], bf16)
                    for uc in range(2):
                        nc.scalar.activation(
                            tanh_t[:, uc, :],
                            score_tiles.pop((g, uc))[:],
                            Tanh,
                            bias=wb_sb[:, ds(uc, 1)],
                        )
                    q = z_pool.tile([128, TT], bf16, tag="q")
                    nc.vector.tensor_scalar_mul(q[:], tanh_t[:, 0, :], v_sb[:, ds(0, 1)])
                    zt = z_pool.tile([128, TT], bf16, tag="z")
                    nc.vector.tensor_scalar_mul(zt[:], tanh_t[:, 1, :], v_sb[:, ds(1, 1)])
                    nc.vector.tensor_add(zt[:], zt[:], q[:])
                    z_tiles[g] = zt

                # ---- ACT: exp once per sample (after last tile's V-dot) ----
                if 0 <= jl < NG and jl % N_TILES == N_TILES - 1:
                    sj = jl // N_TILES
                    lg = logit_tiles.pop(sj)
                    wx = wexp_pool.tile([128, N_TILES * CH], bf16, tag="wexp")
                    nc.scalar.activation(wx[:], lg[:], Exp)
                    wexp_tiles[sj] = wx

                # ---- finalize sample after its last wsum chunk ----
                if 0 <= jw < NG and jw % N_TILES == N_TILES - 1:
                    sj = jw // N_TILES
                    del wexp_tiles[sj]
                    c_ps = c_tiles.pop(sj)
                    recip = fin_pool.tile([1, 1], f32, tag="recip")
                    nc.vector.reciprocal(recip[:], c_ps[0:1, D : D + 1])
                    row = fin_pool.tile([1, D], f32, tag="row")
                    nc.vector.tensor_scalar_mul(row[:], c_ps[0:1, 0:D], recip[:])
                    nc.scalar.dma_start(out_d[ds(sj, 1), :], row[:])

    nc.compile()
    return nc


def _prep_inputs(inputs, W_w, W_b, V_w, V_b):
    x = np.asarray(inputs, dtype=np.float32)
    ones = np.ones((B, T, 1), dtype=np.float32)
    xn_full = np.concatenate([x, ones], axis=2).astype(BF16)      # [B, T, D+1]
    xT_full = np.ascontiguousarray(x.transpose(0, 2, 1)).astype(BF16)  # [B, D, T]

    w = np.asarray(W_w, dtype=np.float32).astype(BF16)            # [D, U]
    wb = np.asarray(W_b, dtype=np.float32).reshape(U // 128, 128).T.copy()  # [128, 2]
    v = np.asarray(V_w, dtype=np.float32).reshape(U // 128, 128).T.copy()  # [128, 2]

    in_maps = []
    for c in range(N_CORES):
        sl = slice(c * S, (c + 1) * S)
        in_maps.append(
            {
                "xT": np.ascontiguousarray(xT_full[sl]),
                "xn": np.ascontiguousarray(xn_full[sl]),
                "w": w,
                "wb": wb,
                "v": v,
            }
        )
    return in_maps


def kernel(inputs, W_w, W_b, V_w, V_b):
    from concourse.bass_utils import run_bass_kernel_spmd

    if "nc" not in _CACHE:
        _CACHE["nc"] = _build()
    nc = _CACHE["nc"]

    in_maps = _prep_inputs(inputs, W_w, W_b, V_w, V_b)
    res = run_bass_kernel_spmd(nc, in_maps, core_ids=list(range(N_CORES)))
    out = np.concatenate([r["out"] for r in res.results], axis=0)
    return np.asarray(out, dtype=np.float32)
